# revision 84
# baseline (speedup 1.0000x reference)
"""Bahdanau encoder-decoder LSTM on 8 Trainium2 NeuronCores.

Strategy: data-parallel over batch (B=32 -> 4 rows per core), weights
replicated, zero collectives. Each core runs the full encoder (T=512
steps) and decoder (512 steps) for its 4 batch rows out of SBUF.

All matmuls are bf16 x bf16 with fp32 PSUM accumulation. Gate/softmax
arithmetic is fp32 on ACT/DVE. Sigmoid is computed as 0.5*tanh(x/2)+0.5
so one ACT table set serves the whole kernel. The LSTM cell state is
kept doubled (S = 2c) and the hidden doubled (H2 = 2h); the 0.5 factors
fold into ACT scales, the transposed-h copies, and a final host-side
0.5 on the DMA'd output.

Scheduling (from perfetto trace analysis of the baseline):
- dscore is computed directly transposed (Wdec as stationary operand)
  so the attention tanh starts ~4us earlier.
- The gate h-pass matmuls are emitted after dscore so the PE streams
  them inside the attention-tanh window; V-reduction matmuls are
  interleaved per batch row right behind their tanh pair.
- Bias matmuls open each z accumulation group and fire during the
  previous step's LSTM tail; tiny data-dependent dummy matmuls keep
  the PE HAM activity monitor from re-throttling the clock to 1.2GHz
  during the tail (the baseline ran ~half the step at half clock).
- ctx-pass matmuls are grouped per gate (f,i,g,o) so the tail's ACT
  ops start while the PE is still streaming the later gates.
"""
import numpy as np
import ml_dtypes

import concourse.bass as bass
import concourse.tile as tile_mod
from concourse import mybir
from concourse.bass_utils import run_bass_kernel_spmd
from concourse.tile import TileContext
from concourse.vector_clock import ScopedClock

F32 = mybir.dt.float32
BF16 = mybir.dt.bfloat16
AF = mybir.ActivationFunctionType
OP = mybir.AluOpType
bf16 = ml_dtypes.bfloat16

F, HE, HD, A = 128, 512, 512, 256
B, T = 32, 512
NCORES = 8
BL = B // NCORES  # 4 batch rows per core
G = 4 * HE        # 2048
GORD = [1, 0, 2, 3]  # gate emission order f, i, g, o (torch i,f,g,o slices)

# ----------------------------------------------------------------------
# Toolchain workarounds: this walrus build refuses any TPB instruction
# carrying more than one semaphore wait. Hoist extras onto standalone
# EventSemaphore instructions (engine program order keeps semantics).
_TPB_ENGINES = None


def _tpb_engines():
    global _TPB_ENGINES
    if _TPB_ENGINES is None:
        _TPB_ENGINES = {
            mybir.EngineType.PE,
            mybir.EngineType.DVE,
            mybir.EngineType.Activation,
            mybir.EngineType.Pool,
            mybir.EngineType.SP,
        }
    return _TPB_ENGINES


def split_multi_waits(nc, cap=1):
    fn = nc.m.functions[0]
    engines = _tpb_engines()
    for bb in fn.blocks:
        insts = bb.instructions
        out = []
        changed = False
        for inst in insts:
            si = inst.sync_info
            waits = None if si is None else si.on_wait
            if waits is not None and len(waits) > cap and inst.engine in engines:
                extra = list(waits[cap:])
                si.on_wait = list(waits[:cap])
                for j, w in enumerate(extra):
                    ev = mybir.InstEventSemaphore(
                        name=f"{inst.name}-xw{j}", ins=[], outs=[]
                    )
                    ev.engine = inst.engine
                    ev.sync_info = mybir.SyncInfo(on_wait=[w], on_update=[])
                    out.append(ev)
                changed = True
            out.append(inst)
        if changed:
            bb.instructions = out


_patched = False


def patch_tile_drain():
    """Same walrus limitation for the Tile tail drain."""
    global _patched
    if _patched:
        return
    _patched = True

    def _drain(self, tick_clock, wait_clock):
        drain_inst = self.nc.sync.drain()
        wait_clock.add_sem_waits(
            drain_inst.ins, ScopedClock({None: tick_clock.global_clock})
        )
        si = drain_inst.ins.sync_info
        if si is not None and si.on_wait is not None and len(si.on_wait) > 1:
            extra = list(si.on_wait[1:])
            si.on_wait = [si.on_wait[0]]
            for w in extra:
                n2 = self.nc.sync.nop()
                n2.ins.sync_info = mybir.SyncInfo(on_wait=[w], on_update=[])
        self.nc.all_engine_barrier()
        popped = self.nc._tile_sem_poison_stack.pop()
        assert popped is self._sem_poison
        self.nc.clear_and_free_semaphores(list(self.sems.allocated().values()))
        self.nc.all_engine_barrier()

    tile_mod.TileContext._drain_and_barrier = _drain


# ----------------------------------------------------------------------
def build_nc(nt=T):
    """Build the single-core Bass program (SPMD across 8 cores)."""
    patch_tile_drain()
    nc = bass.Bass("TRN2", target_bir_lowering=False, debug=False)

    # ---- DRAM parameters (per-core inputs prepared on the host) ----
    d_xT = nc.dram_tensor("xT", [128, nt * BL], BF16, kind="ExternalInput")
    d_whhT = nc.dram_tensor("whhT", [4, 128, G], BF16, kind="ExternalInput")
    d_wihT = nc.dram_tensor("wihT", [128, G], BF16, kind="ExternalInput")
    d_encb = nc.dram_tensor("encb", [1, G], BF16, kind="ExternalInput")
    d_dhr = nc.dram_tensor("dhr", [4, 128, G], BF16, kind="ExternalInput")
    d_dcr = nc.dram_tensor("dcr", [4, 128, G], BF16, kind="ExternalInput")
    d_wdT = nc.dram_tensor("wdT", [4, 128, A], BF16, kind="ExternalInput")
    d_decb = nc.dram_tensor("decb", [1, G], BF16, kind="ExternalInput")
    d_wencT = nc.dram_tensor("wencT", [4, 128, A], BF16, kind="ExternalInput")
    d_epb = nc.dram_tensor("epb", [2, 128, 1], F32, kind="ExternalInput")
    d_vblk = nc.dram_tensor("vblk", [128, 8 * BL], BF16, kind="ExternalInput")
    d_ones = nc.dram_tensor("ones", [1, BL], BF16, kind="ExternalInput")
    d_onesr = nc.dram_tensor("onesr", [1, 128], BF16, kind="ExternalInput")
    d_id4 = nc.dram_tensor("id4", [BL, BL], F32, kind="ExternalInput")
    d_id4h = nc.dram_tensor("id4h", [BL, BL], F32, kind="ExternalInput")
    d_id128 = nc.dram_tensor("id128", [128, 128], BF16, kind="ExternalInput")
    d_hd0T = nc.dram_tensor("hd0T", [128, 4, BL], BF16, kind="ExternalInput")
    d_S0 = nc.dram_tensor("S0", [BL, HD], F32, kind="ExternalInput")
    d_out = nc.dram_tensor("out", [BL, nt, HD], F32, kind="ExternalOutput")

    from contextlib import ExitStack
    with TileContext(nc) as tc, ExitStack() as ctx:
        const = ctx.enter_context(tc.tile_pool(name="const", bufs=1))
        state = ctx.enter_context(tc.tile_pool(name="state", bufs=1))
        work = ctx.enter_context(tc.tile_pool(name="work", bufs=1))

        # ---- load constants into SBUF ----
        xT = const.tile([128, nt * BL], BF16)
        nc.sync.dma_start(out=xT, in_=d_xT.ap())
        whhT = const.tile([128, 4 * G], BF16)
        for k in range(4):
            nc.sync.dma_start(out=whhT[:, k * G:(k + 1) * G], in_=d_whhT.ap()[k])
        wihT = const.tile([128, G], BF16)
        nc.sync.dma_start(out=wihT, in_=d_wihT.ap())
        encb = const.tile([1, G], BF16)
        nc.sync.dma_start(out=encb, in_=d_encb.ap())
        dhr = const.tile([128, 4 * G], BF16)
        for k in range(4):
            nc.sync.dma_start(out=dhr[:, k * G:(k + 1) * G], in_=d_dhr.ap()[k])
        dcr = const.tile([128, 4 * G], BF16)
        for k in range(4):
            nc.sync.dma_start(out=dcr[:, k * G:(k + 1) * G], in_=d_dcr.ap()[k])
        wdT = const.tile([128, 4 * A], BF16)
        for k in range(4):
            nc.sync.dma_start(out=wdT[:, k * A:(k + 1) * A], in_=d_wdT.ap()[k])
        decb = const.tile([1, G], BF16)
        nc.sync.dma_start(out=decb, in_=d_decb.ap())
        wencT = const.tile([128, 4 * A], BF16)
        for k in range(4):
            nc.sync.dma_start(out=wencT[:, k * A:(k + 1) * A], in_=d_wencT.ap()[k])
        epb = const.tile([128, 2], F32)
        for k in range(2):
            nc.sync.dma_start(out=epb[:, k:k + 1], in_=d_epb.ap()[k])
        vblk = const.tile([128, 8 * BL], BF16)
        nc.sync.dma_start(out=vblk, in_=d_vblk.ap())
        ones = const.tile([1, BL], BF16)
        nc.sync.dma_start(out=ones, in_=d_ones.ap())
        onesr = const.tile([1, 128], BF16)
        nc.sync.dma_start(out=onesr, in_=d_onesr.ap())
        id4 = const.tile([BL, BL], F32)
        nc.sync.dma_start(out=id4, in_=d_id4.ap())
        id4h = const.tile([BL, BL], F32)
        nc.sync.dma_start(out=id4h, in_=d_id4h.ap())
        id128 = const.tile([128, 128], BF16)
        nc.sync.dma_start(out=id128, in_=d_id128.ap())

        # ---- persistent state ----
        encT = state.tile([128, 4, nt * BL], BF16)    # [e, ec, (t, b)]
        S_enc = state.tile([BL, HE], F32)             # 2*c_enc
        nc.vector.memset(S_enc, 0.0)
        hdT = state.tile([128, 4, BL], BF16)          # [k, kc, b] = h
        nc.sync.dma_start(out=hdT, in_=d_hd0T.ap())
        S_dec = state.tile([BL, HD], F32)             # 2*c_dec
        nc.sync.dma_start(out=S_dec, in_=d_S0.ap())
        # transposed softmax weights [128, pair, tc, slot]: w(2p) in slot 0,
        # w(2p+1) in slot 3, slots 1/2 stay zero. A stride-2 [128,2] slice
        # at slot s then reads (w, 0) or (0, w) -- a one-hot stationary that
        # drops ctx(b) into psum row b%2 of the pair tile.
        wTb = state.tile([128, 2, 4, 4], BF16)
        nc.vector.memset(wTb, 0.0)
        ctxT = state.tile([128, 4, BL], BF16)         # [k, kc, b]
        enc_projT = state.tile([128, 2 * nt * BL], BF16)  # [a, (ac, b, t)]
        enc_tb = state.tile([128, 16 * HE], BF16)        # [t', (b, tc, e)]
        nc.vector.memset(enc_tb, 0.0)
        # 4-slot align ring: slot (b%2)*2+ac; pairs alternate slots so only
        # one pair's tanh outputs are live at a time.
        align = state.tile([128, 4 * T], BF16)

        # ---- per-step work tiles ----
        t_i = work.tile([BL, HE], F32)
        t_f = work.tile([BL, HE], F32)
        t_g = work.tile([BL, HE], F32)
        t_o = work.tile([BL, HE], F32)
        toT = work.tile([128, 4, BL], F32)  # 0.5*tanh(o) transposed (SBUF)
        fS = work.tile([BL, HE], F32)
        b2 = work.tile([BL, HE], F32)
        th = work.tile([BL, HE], F32)
        H2 = work.tile([BL, HE], F32)
        dsT = work.tile([128, 2 * BL], F32)
        w_u = [work.tile([2, T], F32, name=f"wu{p}") for p in range(2)]
        denom = [work.tile([2, 1], F32, name=f"dn{p}") for p in range(2)]
        rcp = [work.tile([2, 1], F32, name=f"rc{p}") for p in range(2)]
        cs = [work.tile([2, HE], F32, name=f"cs{p}") for p in range(2)]

        def lstm_tail(zt, S, ptb, hT_out, dummy_mm=None, emit_H2=False):
            """Gates + state update. zt = per-gate psum tiles; S = 2c.

            3-op DVE chain: fS = (tf+1)*S = 2fS; b2 = (ti+1)*tg = 2ig;
            S_new = 0.5*fS + b2 = 2c_new. The transposed hidden for step
            t+1 is built directly from transposed tanh(o)/tanh(c) tiles:
            hT = (0.5*to + 0.5) * thT, with the 0.5*to coming from a
            0.5-scaled identity in the transpose. H2 (= 2h, for the output
            DMA) is computed off the recurrence chain.

            ptb: psum staging tile [128, 2, 4, BL] -- region [:,0] takes
            thT, region [:,1] takes 0.5*toT (reused from the attention
            staging, drained by then). hT_out(h) -> [128, 2, BL] dst slice.
            """
            tt = {0: t_i, 1: t_f, 2: t_g, 3: t_o}
            for ng in GORD:
                sc = 1.0 if ng == 2 else 0.5
                nc.scalar.activation(out=tt[ng], in_=zt[ng],
                                     func=AF.Tanh, bias=0.0, scale=sc)
            # fS emitted first: only depends on tanh_f (first tanh in GORD)
            nc.vector.scalar_tensor_tensor(out=fS, in0=t_f, scalar=1.0, in1=S,
                                           op0=OP.add, op1=OP.mult)
            if dummy_mm is not None:
                dummy_mm(t_f)
                dummy_mm(t_g)
            # b2 = (ti+1)*tg = 2ig
            nc.vector.scalar_tensor_tensor(out=b2, in0=t_i, scalar=1.0, in1=t_g,
                                           op0=OP.add, op1=OP.mult)
            if dummy_mm is not None:
                dummy_mm(fS)
            # 0.5*to transposed (off-chain: t_o ready right after its group);
            # staged to SBUF since the hT stt may read only one PSUM input.
            # The 0.5 folds into the staging copy (HW transpose mode does
            # not multiply by the identity's values).
            for ec in range(4):
                nc.tensor.transpose(ptb[:, 1, ec],
                                    t_o[:, ec * 128:(ec + 1) * 128], id4)
            nc.vector.tensor_scalar(out=toT, in0=ptb[:, 1], scalar1=0.5,
                                    scalar2=None, op0=OP.mult)
            for h in range(2):
                c = slice(h * 256, (h + 1) * 256)
                # S_new = 0.5*fS + b2 = 2c_new
                nc.vector.scalar_tensor_tensor(
                    out=S[:, c], in0=fS[:, c], scalar=0.5, in1=b2[:, c],
                    op0=OP.mult, op1=OP.add)
                nc.scalar.activation(out=th[:, c], in_=S[:, c], func=AF.Tanh,
                                     bias=0.0, scale=0.5)
                for ec in (2 * h, 2 * h + 1):
                    nc.tensor.transpose(ptb[:, 0, ec],
                                        th[:, ec * 128:(ec + 1) * 128], id4)
                # hT half = (0.5*to + 0.5) * th, transposed
                nc.vector.scalar_tensor_tensor(
                    out=hT_out(h), in0=toT[:, 2 * h:2 * h + 2], scalar=0.5,
                    in1=ptb[:, 0, 2 * h:2 * h + 2], op0=OP.add, op1=OP.mult)
            if dummy_mm is not None:
                dummy_mm(th)
            if emit_H2:
                for h in range(2):
                    c = slice(h * 256, (h + 1) * 256)
                    # H2 = (to+1)*th = 2h (output DMA only; off-chain)
                    nc.vector.scalar_tensor_tensor(
                        out=H2[:, c], in0=t_o[:, c], scalar=1.0, in1=th[:, c],
                        op0=OP.add, op1=OP.mult)

        # ===== PHASE 0: xwb = x @ Wih.T + enc_bias for all t (bf16) =====
        xwb = state.tile([128, 16 * G], BF16)
        with tc.tile_pool(name="p0", bufs=4, space="PSUM") as p0:
            for c in range(16):
                for ng in range(4):
                    sl = slice(ng * HE, (ng + 1) * HE)
                    pp0 = p0.tile([128, HE], F32, tag="pp0")
                    nc.tensor.matmul(pp0, onesr, encb[:, sl],
                                     start=True, stop=False)
                    nc.tensor.matmul(pp0, xT[:, 128 * c:128 * (c + 1)],
                                     wihT[:, sl], start=False, stop=True)
                    dst = xwb[:, c * G + ng * HE: c * G + (ng + 1) * HE]
                    if ng % 2 == 0:
                        nc.vector.tensor_copy(dst, pp0)
                    else:
                        nc.scalar.copy(dst, pp0)

        # ================= ENCODER =================
        with tc.tile_pool(name="eps", bufs=1, space="PSUM") as eps, \
             tc.tile_pool(name="ept", bufs=1, space="PSUM") as ept, \
             tc.tile_pool(name="edum", bufs=1, space="PSUM") as edum:
            dum_e = edum.tile([BL, 16], F32)
            # one PSUM bank staging the thT/toT transposes
            eptb = ept.tile([128, 2, 4, BL], F32)

            def enc_dummy(src):
                nc.tensor.matmul(dum_e, id4, src[:, 0:16], start=True, stop=True)

            def enc_inject(t):
                # xwb inject opens each z group; emitted at the END of step
                # t-1 so it schedules into the tail's PE idle (it has no
                # encT dependency, only the zt WAR on the gate tanh reads).
                zt = {ng: eps.tile([BL, HE], F32, tag=f"z{ng}", name=f"ez{ng}_{t}")
                      for ng in GORD}
                c0, r0 = t // 32, (t % 32) * BL
                for ng in GORD:
                    nc.tensor.matmul(zt[ng], id128[:, r0:r0 + BL],
                                     xwb[:, c0 * G + ng * HE: c0 * G + (ng + 1) * HE],
                                     start=True, stop=(t == 0))
                return zt

            zt = enc_inject(0)
            for t in range(nt):
                if t > 0:
                    for ng in GORD:
                        sl = slice(ng * HE, (ng + 1) * HE)
                        for kc in range(4):
                            nc.tensor.matmul(
                                zt[ng],
                                encT[:, kc, BL * (t - 1): BL * t],
                                whhT[:, kc * G + ng * HE: kc * G + (ng + 1) * HE],
                                start=False, stop=(kc == 3))
                # hT halves land straight into encT; the next step's h-pass
                # kc0/1 matmuls only RAW-wait on their own half
                lstm_tail(zt, S_enc, eptb,
                          lambda h, _t=t: encT[:, 2 * h:2 * h + 2,
                                               BL * _t: BL * (_t + 1)],
                          dummy_mm=enc_dummy)
                if t + 1 < nt:
                    zt = enc_inject(t + 1)

        # ============ PHASE 2: enc_projT and enc_tb ============
        with tc.tile_pool(name="p2a", bufs=4, space="PSUM") as p2a, \
             tc.tile_pool(name="p2b", bufs=4, space="PSUM") as p2b:
            for ac in range(2):
                for b in range(BL):
                    pp = p2a.tile([128, nt], F32, tag="pp")
                    for ec in range(4):
                        rhs = encT[:, ec, b: b + BL * (nt - 1) + 1: BL]
                        nc.tensor.matmul(
                            pp,
                            wencT[:, ec * A + ac * 128: ec * A + (ac + 1) * 128],
                            rhs, start=(ec == 0), stop=(ec == 3))
                    nc.scalar.activation(
                        out=enc_projT[:, ac * (nt * BL) + b * nt:
                                      ac * (nt * BL) + (b + 1) * nt],
                        in_=pp, func=AF.Identity, bias=epb[:, ac:ac + 1], scale=1.0)
            # enc_tb via PE transposes of encT (transpose keeps dtype)
            for b in range(BL):
                for tc_i in range(nt // 128):
                    for ec in range(4):
                        base = (128 * tc_i) * BL + b
                        src = encT[:, ec, base: base + BL * 127 + 1: BL]
                        pt2 = p2b.tile([128, 128], BF16, tag="pt2")
                        nc.tensor.transpose(pt2, src, id128)
                        nc.vector.tensor_copy(
                            enc_tb[:, (b * (nt // 128) + tc_i) * HE + ec * 128:
                                   (b * (nt // 128) + tc_i) * HE + (ec + 1) * 128],
                            pt2)

        # ================= DECODER =================
        with tc.tile_pool(name="dz", bufs=1, space="PSUM") as dz, \
             tc.tile_pool(name="dsp", bufs=1, space="PSUM") as dsp, \
             tc.tile_pool(name="dlc", bufs=1, space="PSUM") as dlc, \
             tc.tile_pool(name="dpt", bufs=1, space="PSUM") as dpt:
            # one PSUM bank holding 4 rotating [128, BL] transpose slots
            # one PSUM bank staging every transpose group; each group is
            # drained by one (or two) strided DVE copies, so the PE never
            # ping-pongs with the DVE on single-tile WAR chains.
            # region [:, 0]    : hdT staging  (ec, b)
            # region [:, 1, :, 0:2]: w-pair staging (tc, b%2)
            # region [:, 1, :, 2:4]: ctxT-pair staging (ec, b%2)
            dptb = dpt.tile([128, 2, 4, BL], F32)

            def align_op(b, ac):
                s = (b % 2) * 2 + ac
                nc.scalar.activation(
                    out=align[:, s * T: s * T + nt],
                    in_=enc_projT[:, ac * (nt * BL) + b * nt:
                                  ac * (nt * BL) + (b + 1) * nt],
                    func=AF.Tanh, bias=dsT[:, ac * BL + b: ac * BL + b + 1],
                    scale=1.0)

            def vred_op(lg_ps, b, ac):
                # lhsT [128,2] has V(ac) at col b%2, zeros at the other, so
                # logits(b) land in pair row b%2; the pair's 4 mms form one
                # accumulation group (rows take turns getting +0).
                col = (b * 2 + ac) * BL + (b & ~1)
                s = (b % 2) * 2 + ac
                nc.tensor.matmul(
                    lg_ps[:, :nt],
                    vblk[:, col:col + 2],
                    align[:, s * T: s * T + nt],
                    start=(b % 2 == 0 and ac == 0), stop=(b % 2 == 1 and ac == 1))

            def hp_op(zt, ng):
                for kc in range(4):
                    nc.tensor.matmul(
                        zt[ng],
                        hdT[:, kc],
                        dhr[:, kc * G + ng * HD: kc * G + (ng + 1) * HD],
                        start=False, stop=False)

            def exp_op(lg_ps, p):
                nc.scalar.activation(
                    out=w_u[p][:, :nt],
                    in_=lg_ps[:, :nt], func=AF.Exp,
                    bias=0.0, scale=1.0, accum_out=denom[p])

            def wtr_op(p):
                # transpose the pair's (unnormalized) softmax rows into the
                # staging bank, then two strided copies into wTb slots 0/3
                for tc_i in range(nt // 128):
                    nc.tensor.transpose(
                        dptb[:, 1, tc_i, 0:2],
                        w_u[p][:, tc_i * 128:(tc_i + 1) * 128],
                        id4[0:2, 0:2])
                nc.vector.tensor_copy(wTb[:, p, :, 0:4:3], dptb[:, 1, :, 0:2])

            def ctx_op(ctx_ps, p, rows=(0, 1)):
                # one accumulation group per pair; rows may be emitted in
                # two chunks with other matmuls interleaved between
                for s in rows:
                    b = 2 * p + s
                    for tc_i in range(nt // 128):
                        base = (b * (nt // 128) + tc_i) * HE
                        nc.tensor.matmul(
                            ctx_ps[:, :HE],
                            wTb[:, p, tc_i, s: s + 3: 2],
                            enc_tb[:, base: base + HE],
                            start=(s == 0 and tc_i == 0),
                            stop=(s == 1 and tc_i == 3))

            def norm_op(ctx_ps, p, split=False):
                if split:
                    # halves in parallel on DVE + ACT (~400ns vs 658)
                    nc.vector.tensor_scalar(out=cs[p][:, 0:HE // 2],
                                            in0=ctx_ps[:, 0:HE // 2],
                                            scalar1=rcp[p],
                                            scalar2=None, op0=OP.mult)
                    nc.scalar.mul(cs[p][:, HE // 2:HE], ctx_ps[:, HE // 2:HE],
                                  rcp[p])
                    return
                nc.vector.tensor_scalar(out=cs[p], in0=ctx_ps[:, :HE],
                                        scalar1=rcp[p],
                                        scalar2=None, op0=OP.mult)

            def ctxT_op(p, h=None):
                ecs = range(4) if h is None else (2 * h, 2 * h + 1)
                for ec in ecs:
                    nc.tensor.transpose(
                        dptb[:, 1, ec, 2:4], cs[p][:, ec * 128:(ec + 1) * 128],
                        id4[0:2, 0:2])
                e0, e1 = (0, 4) if h is None else (2 * h, 2 * h + 2)
                nc.vector.tensor_copy(ctxT[:, e0:e1, 2 * p:2 * p + 2],
                                      dptb[:, 1, e0:e1, 2:4])

            def dec_bias(t):
                # bias matmuls open each z group; emitted at the END of step
                # t-1 so they schedule into the tail's PE idle.
                zt = {ng: dz.tile([BL, HD], F32, tag=f"zd{ng}", name=f"dz{ng}_{t}")
                      for ng in GORD}
                for ng in GORD:
                    nc.tensor.matmul(zt[ng], ones, decb[:, ng * HD:(ng + 1) * HD],
                                     start=True, stop=False)
                return zt

            zt = dec_bias(0)
            for t in range(nt):
                dsT_ps = dsp.tile([128, 2 * BL], F32, tag="dsT")
                lg_ps = dlc.tile([2, T], F32, tag="lg")
                ctx_ps = dlc.tile([2, T], F32, tag="cx")

                def dec_dummy(src, _p=dsT_ps):
                    nc.tensor.matmul(_p[0:BL, 0:8], id4, src[:, 0:8],
                                     start=True, stop=True)

                # --- dscore, directly transposed: dsT = Wdec @ h ---
                for ac in range(2):
                    for kc in range(4):
                        nc.tensor.matmul(
                            dsT_ps[:, ac * BL:(ac + 1) * BL],
                            wdT[:, kc * A + ac * 128: kc * A + (ac + 1) * 128],
                            hdT[:, kc],
                            start=(kc == 0), stop=(kc == 3))
                nc.vector.tensor_copy(dsT, dsT_ps)
                # --- attention pipelined per pair of batch rows; the h-pass
                # matmuls stream inside the align-tanh window. Pair 0's
                # softmax/ctx/normalize/ctxT all run inside pair 1's align
                # window; only pair 1's trail is on the serial chain. ---
                for b in (0, 1):
                    align_op(b, 0)
                    align_op(b, 1)
                hp_op(zt, GORD[0])
                vred_op(lg_ps, 0, 0)
                vred_op(lg_ps, 0, 1)
                hp_op(zt, GORD[1])
                vred_op(lg_ps, 1, 0)
                vred_op(lg_ps, 1, 1)
                exp_op(lg_ps, 0)
                nc.vector.reciprocal(rcp[0], denom[0])
                for b in (2, 3):
                    align_op(b, 0)
                    align_op(b, 1)
                vred_op(lg_ps, 2, 0)
                vred_op(lg_ps, 2, 1)
                wtr_op(0)
                ctx_op(ctx_ps, 0, rows=(0,))
                vred_op(lg_ps, 3, 0)
                vred_op(lg_ps, 3, 1)
                ctx_op(ctx_ps, 0, rows=(1,))
                exp_op(lg_ps, 1)
                nc.vector.reciprocal(rcp[1], denom[1])
                wtr_op(1)
                norm_op(ctx_ps, 0)
                ctx_op(ctx_ps, 1)
                ctxT_op(0)
                hp_op(zt, GORD[2])
                hp_op(zt, GORD[3])
                norm_op(ctx_ps, 1)
                ctxT_op(1)
                # --- ctx-pass, grouped per gate; tail follows early ---
                for ng in GORD:
                    sl = slice(ng * HD, (ng + 1) * HD)
                    for kc in range(4):
                        nc.tensor.matmul(
                            zt[ng],
                            ctxT[:, kc],
                            dcr[:, kc * G + ng * HD: kc * G + (ng + 1) * HD],
                            start=False, stop=(kc == 3))
                # --- gates + state; transposed h lands straight in hdT ---
                lstm_tail(zt, S_dec, dptb, lambda h: hdT[:, 2 * h:2 * h + 2],
                          dummy_mm=dec_dummy, emit_H2=True)
                # --- output h (as 2h; host rescales) ---
                nc.sync.dma_start(out=d_out.ap()[:, t, :], in_=H2)
                if t + 1 < nt:
                    zt = dec_bias(t + 1)

    split_multi_waits(nc)
    return nc


# ----------------------------------------------------------------------
def _sig(x):
    return 1.0 / (1.0 + np.exp(-x))


def prepare_inputs(inputs, nt=T):
    """Host-side weight/layout prep. Returns per_core_fn."""
    f32 = np.float32
    enc_Wih = np.asarray(inputs["enc_Wih"], f32)
    enc_Whh = np.asarray(inputs["enc_Whh"], f32)
    enc_bias = np.asarray(inputs["enc_bih"], f32) + np.asarray(inputs["enc_bhh"], f32)
    Wenc_w = np.asarray(inputs["Wenc_w"], f32)
    Wenc_b = np.asarray(inputs["Wenc_b"], f32)
    Wdec_w = np.asarray(inputs["Wdec_w"], f32)
    Wdec_b = np.asarray(inputs["Wdec_b"], f32)
    V_w = np.asarray(inputs["V_w"], f32)
    attn_bias = np.asarray(inputs["attn_bias"], f32)
    dec_Wih = np.asarray(inputs["dec_Wih"], f32)
    dec_Whh = np.asarray(inputs["dec_Whh"], f32)
    dec_bias = np.asarray(inputs["dec_bih"], f32) + np.asarray(inputs["dec_bhh"], f32)

    sh = {}
    sh["whhT"] = np.ascontiguousarray(
        enc_Whh.T.reshape(4, 128, G)).astype(bf16)
    sh["wihT"] = np.ascontiguousarray(enc_Wih.T).astype(bf16)
    sh["encb"] = enc_bias.reshape(1, -1).astype(bf16)
    dec_h_w = (dec_Wih[:, HD:] + dec_Whh)           # [2048, 512]
    sh["dhr"] = np.ascontiguousarray(
        dec_h_w.T.reshape(4, 128, G)).astype(bf16)
    sh["dcr"] = np.ascontiguousarray(
        dec_Wih[:, :HD].T.reshape(4, 128, G)).astype(bf16)
    # wdT[kc][k, ac*128+m] = Wdec_w[ac*128+m, kc*128+k]
    sh["wdT"] = np.ascontiguousarray(
        Wdec_w.T.reshape(4, 128, A)).astype(bf16)
    sh["decb"] = dec_bias.reshape(1, -1).astype(bf16)
    sh["wencT"] = np.ascontiguousarray(
        Wenc_w.T.reshape(4, 128, A)).astype(bf16)
    sh["epb"] = np.ascontiguousarray(
        (Wenc_b + attn_bias + Wdec_b).reshape(2, 128, 1)).astype(f32)
    vb = np.zeros((128, 8 * BL), f32)
    for b in range(BL):
        for ac in range(2):
            vb[:, (b * 2 + ac) * BL + b] = V_w[0, ac * 128:(ac + 1) * 128]
    sh["vblk"] = vb.astype(bf16)
    sh["ones"] = np.ones((1, BL), f32).astype(bf16)
    sh["onesr"] = np.ones((1, 128), f32).astype(bf16)
    sh["id4"] = np.eye(BL, dtype=f32)
    sh["id4h"] = 0.5 * np.eye(BL, dtype=f32)
    sh["id128"] = np.eye(128, dtype=f32).astype(bf16)
    # decoder init state (z0 from biases only)
    i0, f0, g0, o0 = np.split(dec_bias, 4)
    cd0 = _sig(i0) * np.tanh(g0)
    hd0 = _sig(o0) * np.tanh(cd0)
    hd0T = np.zeros((128, 4, BL), f32)
    for ec in range(4):
        for b in range(BL):
            hd0T[:, ec, b] = hd0[ec * 128:(ec + 1) * 128]
    sh["hd0T"] = hd0T.astype(bf16)
    sh["S0"] = np.broadcast_to(2.0 * cd0, (BL, HD)).astype(f32).copy()

    x = np.asarray(inputs["x"], f32)

    def core_inputs(core):
        xc = x[core * BL:(core + 1) * BL, :nt, :]      # [BL, nt, F]
        xT = np.ascontiguousarray(xc.transpose(2, 1, 0).reshape(128, nt * BL))
        m = dict(sh)
        m["xT"] = xT.astype(bf16)
        return m

    return core_inputs


_cache = {}


def kernel(**inputs):
    nt = np.asarray(inputs["x"]).shape[1]
    if nt not in _cache:
        _cache[nt] = build_nc(nt)
    nc = _cache[nt]
    core_inputs = prepare_inputs(inputs, nt)
    in_maps = [core_inputs(c) for c in range(NCORES)]
    res = run_bass_kernel_spmd(nc, in_maps, core_ids=list(range(NCORES)))
    outs = [res.results[c]["out"] for c in range(NCORES)]
    full = np.concatenate(outs, axis=0) * 0.5
    return full.astype(np.float32)



# revision 88
# speedup vs baseline: 1.1008x; 1.1008x over previous
"""Bahdanau encoder-decoder LSTM on 8 Trainium2 NeuronCores.

Strategy: data-parallel over batch (B=32 -> 4 rows per core), weights
replicated, zero collectives. Each core runs the full encoder (T=512
steps) and decoder (512 steps) for its 4 batch rows out of SBUF.

All matmuls are bf16 x bf16 with fp32 PSUM accumulation. Gate/softmax
arithmetic is fp32 on ACT/DVE. Sigmoid is computed as 0.5*tanh(x/2)+0.5
so one ACT table set serves the whole kernel. The LSTM cell state is
kept doubled (S = 2c) and the hidden doubled (H2 = 2h); the 0.5 factors
fold into ACT scales, the transposed-h staging, and a final host-side
0.5 on the DMA'd output.

Scheduling (driven by cost-model timeline analysis; ~12.5 ms modeled
vs ~14 ms for the previous version):
- The encoder x-projection + bias is precomputed once as a real GEMM
  (phase 0) and injected per step through an identity-slice stationary,
  removing the per-step bias matmuls from the encoder.
- The decoder attention is pipelined over pairs of batch rows: pair 0's
  softmax/transpose/ctx/normalize/ctxT all execute inside pair 1's
  align-tanh window, so only pair 1's trail is on the serial chain.
  V-reductions accumulate into [2,T] pair tiles via 2-col one-hot
  stationaries (PE psum writes must start at partition 0/32/64/96).
- All small transposes stage into disjoint regions of one PSUM bank and
  drain with single strided DVE copies; per-group batched copies avoid
  the PE<->DVE ping-pong that tile-granular WAR hazards otherwise force.
- The next step's transposed hidden (hdT / encT column) is built
  directly from transposed tanh(c)/tanh(o) tiles:
  hT = (0.5*toT + 0.5) * thT, so H2 and a PSUM round-trip drop off the
  recurrence-critical chain (H2 is still produced for the output DMA).
  NOTE: the HW transpose mode does not multiply by the identity's
  values, so scale factors must fold into ACT/DVE ops, never the
  transpose identity.
- The LSTM tail is a 3-op DVE chain (fS=(tf+1)S, b2=(ti+1)tg,
  S'=0.5 fS+b2) with the trailing S/tanh ops half-chunked so ACT and
  DVE pipeline; dummy matmuls keep the PE HAM clock at 2.4 GHz.
"""
import numpy as np
import ml_dtypes

import concourse.bass as bass
import concourse.tile as tile_mod
from concourse import mybir
from concourse.bass_utils import run_bass_kernel_spmd
from concourse.tile import TileContext
from concourse.vector_clock import ScopedClock

F32 = mybir.dt.float32
BF16 = mybir.dt.bfloat16
AF = mybir.ActivationFunctionType
OP = mybir.AluOpType
bf16 = ml_dtypes.bfloat16

F, HE, HD, A = 128, 512, 512, 256
B, T = 32, 512
NCORES = 8
BL = B // NCORES  # 4 batch rows per core
G = 4 * HE        # 2048
GORD = [1, 0, 2, 3]  # gate emission order f, i, g, o (torch i,f,g,o slices)

# ----------------------------------------------------------------------
# Toolchain workarounds: this walrus build refuses any TPB instruction
# carrying more than one semaphore wait. Hoist extras onto standalone
# EventSemaphore instructions (engine program order keeps semantics).
_TPB_ENGINES = None


def _tpb_engines():
    global _TPB_ENGINES
    if _TPB_ENGINES is None:
        _TPB_ENGINES = {
            mybir.EngineType.PE,
            mybir.EngineType.DVE,
            mybir.EngineType.Activation,
            mybir.EngineType.Pool,
            mybir.EngineType.SP,
        }
    return _TPB_ENGINES


def split_multi_waits(nc, cap=1):
    fn = nc.m.functions[0]
    engines = _tpb_engines()
    for bb in fn.blocks:
        insts = bb.instructions
        out = []
        changed = False
        for inst in insts:
            si = inst.sync_info
            waits = None if si is None else si.on_wait
            if waits is not None and len(waits) > cap and inst.engine in engines:
                extra = list(waits[cap:])
                si.on_wait = list(waits[:cap])
                for j, w in enumerate(extra):
                    ev = mybir.InstEventSemaphore(
                        name=f"{inst.name}-xw{j}", ins=[], outs=[]
                    )
                    ev.engine = inst.engine
                    ev.sync_info = mybir.SyncInfo(on_wait=[w], on_update=[])
                    out.append(ev)
                changed = True
            out.append(inst)
        if changed:
            bb.instructions = out


_patched = False


def patch_tile_drain():
    """Same walrus limitation for the Tile tail drain."""
    global _patched
    if _patched:
        return
    _patched = True

    def _drain(self, tick_clock, wait_clock):
        drain_inst = self.nc.sync.drain()
        wait_clock.add_sem_waits(
            drain_inst.ins, ScopedClock({None: tick_clock.global_clock})
        )
        si = drain_inst.ins.sync_info
        if si is not None and si.on_wait is not None and len(si.on_wait) > 1:
            extra = list(si.on_wait[1:])
            si.on_wait = [si.on_wait[0]]
            for w in extra:
                n2 = self.nc.sync.nop()
                n2.ins.sync_info = mybir.SyncInfo(on_wait=[w], on_update=[])
        self.nc.all_engine_barrier()
        popped = self.nc._tile_sem_poison_stack.pop()
        assert popped is self._sem_poison
        self.nc.clear_and_free_semaphores(list(self.sems.allocated().values()))
        self.nc.all_engine_barrier()

    tile_mod.TileContext._drain_and_barrier = _drain


# ----------------------------------------------------------------------
def build_nc(nt=T):
    """Build the single-core Bass program (SPMD across 8 cores)."""
    patch_tile_drain()
    nc = bass.Bass("TRN2", target_bir_lowering=False, debug=False)

    # ---- DRAM parameters (per-core inputs prepared on the host) ----
    d_xT = nc.dram_tensor("xT", [128, nt * BL], BF16, kind="ExternalInput")
    d_whhT = nc.dram_tensor("whhT", [4, 128, G], BF16, kind="ExternalInput")
    d_wihT = nc.dram_tensor("wihT", [128, G], BF16, kind="ExternalInput")
    d_encb = nc.dram_tensor("encb", [1, G], BF16, kind="ExternalInput")
    d_dhr = nc.dram_tensor("dhr", [4, 128, G], BF16, kind="ExternalInput")
    d_dcr = nc.dram_tensor("dcr", [4, 128, G], BF16, kind="ExternalInput")
    d_wdT = nc.dram_tensor("wdT", [4, 128, A], BF16, kind="ExternalInput")
    d_decb = nc.dram_tensor("decb", [1, G], BF16, kind="ExternalInput")
    d_wencT = nc.dram_tensor("wencT", [4, 128, A], BF16, kind="ExternalInput")
    d_epb = nc.dram_tensor("epb", [2, 128, 1], F32, kind="ExternalInput")
    d_vblk = nc.dram_tensor("vblk", [128, 8 * BL], BF16, kind="ExternalInput")
    d_ones = nc.dram_tensor("ones", [1, BL], BF16, kind="ExternalInput")
    d_onesr = nc.dram_tensor("onesr", [1, 128], BF16, kind="ExternalInput")
    d_id4 = nc.dram_tensor("id4", [BL, BL], F32, kind="ExternalInput")
    d_id4h = nc.dram_tensor("id4h", [BL, BL], F32, kind="ExternalInput")
    d_id128 = nc.dram_tensor("id128", [128, 128], BF16, kind="ExternalInput")
    d_hd0T = nc.dram_tensor("hd0T", [128, 4, BL], BF16, kind="ExternalInput")
    d_S0 = nc.dram_tensor("S0", [BL, HD], F32, kind="ExternalInput")
    d_out = nc.dram_tensor("out", [BL, nt, HD], F32, kind="ExternalOutput")

    from contextlib import ExitStack
    with TileContext(nc) as tc, ExitStack() as ctx:
        const = ctx.enter_context(tc.tile_pool(name="const", bufs=1))
        state = ctx.enter_context(tc.tile_pool(name="state", bufs=1))
        work = ctx.enter_context(tc.tile_pool(name="work", bufs=1))

        # ---- load constants into SBUF ----
        xT = const.tile([128, nt * BL], BF16)
        nc.sync.dma_start(out=xT, in_=d_xT.ap())
        whhT = const.tile([128, 4 * G], BF16)
        for k in range(4):
            nc.sync.dma_start(out=whhT[:, k * G:(k + 1) * G], in_=d_whhT.ap()[k])
        wihT = const.tile([128, G], BF16)
        nc.sync.dma_start(out=wihT, in_=d_wihT.ap())
        encb = const.tile([1, G], BF16)
        nc.sync.dma_start(out=encb, in_=d_encb.ap())
        dhr = const.tile([128, 4 * G], BF16)
        for k in range(4):
            nc.sync.dma_start(out=dhr[:, k * G:(k + 1) * G], in_=d_dhr.ap()[k])
        dcr = const.tile([128, 4 * G], BF16)
        for k in range(4):
            nc.sync.dma_start(out=dcr[:, k * G:(k + 1) * G], in_=d_dcr.ap()[k])
        wdT = const.tile([128, 4 * A], BF16)
        for k in range(4):
            nc.sync.dma_start(out=wdT[:, k * A:(k + 1) * A], in_=d_wdT.ap()[k])
        decb = const.tile([1, G], BF16)
        nc.sync.dma_start(out=decb, in_=d_decb.ap())
        wencT = const.tile([128, 4 * A], BF16)
        for k in range(4):
            nc.sync.dma_start(out=wencT[:, k * A:(k + 1) * A], in_=d_wencT.ap()[k])
        epb = const.tile([128, 2], F32)
        for k in range(2):
            nc.sync.dma_start(out=epb[:, k:k + 1], in_=d_epb.ap()[k])
        vblk = const.tile([128, 8 * BL], BF16)
        nc.sync.dma_start(out=vblk, in_=d_vblk.ap())
        ones = const.tile([1, BL], BF16)
        nc.sync.dma_start(out=ones, in_=d_ones.ap())
        onesr = const.tile([1, 128], BF16)
        nc.sync.dma_start(out=onesr, in_=d_onesr.ap())
        id4 = const.tile([BL, BL], F32)
        nc.sync.dma_start(out=id4, in_=d_id4.ap())
        id4h = const.tile([BL, BL], F32)
        nc.sync.dma_start(out=id4h, in_=d_id4h.ap())
        id128 = const.tile([128, 128], BF16)
        nc.sync.dma_start(out=id128, in_=d_id128.ap())

        # ---- persistent state ----
        encT = state.tile([128, 4, nt * BL], BF16)    # [e, ec, (t, b)]
        S_enc = state.tile([BL, HE], F32)             # 2*c_enc
        nc.vector.memset(S_enc, 0.0)
        hdT = state.tile([128, 4, BL], BF16)          # [k, kc, b] = h
        nc.sync.dma_start(out=hdT, in_=d_hd0T.ap())
        S_dec = state.tile([BL, HD], F32)             # 2*c_dec
        nc.sync.dma_start(out=S_dec, in_=d_S0.ap())
        # transposed softmax weights [128, pair, tc, slot]: w(2p) in slot 0,
        # w(2p+1) in slot 3, slots 1/2 stay zero. A stride-2 [128,2] slice
        # at slot s then reads (w, 0) or (0, w) -- a one-hot stationary that
        # drops ctx(b) into psum row b%2 of the pair tile.
        wTb = state.tile([128, 2, 4, 4], BF16)
        nc.vector.memset(wTb, 0.0)
        ctxT = state.tile([128, 4, BL], BF16)         # [k, kc, b]
        enc_projT = state.tile([128, 2 * nt * BL], BF16)  # [a, (ac, b, t)]
        enc_tb = state.tile([128, 16 * HE], BF16)        # [t', (b, tc, e)]
        nc.vector.memset(enc_tb, 0.0)
        # 4-slot align ring: slot (b%2)*2+ac; pairs alternate slots so only
        # one pair's tanh outputs are live at a time.
        align = state.tile([128, 4 * T], BF16)

        # ---- per-step work tiles ----
        t_i = work.tile([BL, HE], F32)
        t_f = work.tile([BL, HE], F32)
        t_g = work.tile([BL, HE], F32)
        t_o = work.tile([BL, HE], F32)
        toT = work.tile([128, 4, BL], F32)  # 0.5*tanh(o) transposed (SBUF)
        fS = work.tile([BL, HE], F32)
        b2 = work.tile([BL, HE], F32)
        th = work.tile([BL, HE], F32)
        H2 = work.tile([BL, HE], F32)
        dsT = work.tile([128, 2 * BL], F32)
        w_u = [work.tile([2, T], F32, name=f"wu{p}") for p in range(2)]
        denom = [work.tile([2, 1], F32, name=f"dn{p}") for p in range(2)]
        rcp = [work.tile([2, 1], F32, name=f"rc{p}") for p in range(2)]
        cs = [work.tile([2, HE], F32, name=f"cs{p}") for p in range(2)]

        def lstm_tail(zt, S, ptb, hT_out, dummy_mm=None, emit_H2=False):
            """Gates + state update. zt = per-gate psum tiles; S = 2c.

            3-op DVE chain: fS = (tf+1)*S = 2fS; b2 = (ti+1)*tg = 2ig;
            S_new = 0.5*fS + b2 = 2c_new. The transposed hidden for step
            t+1 is built directly from transposed tanh(o)/tanh(c) tiles:
            hT = (0.5*to + 0.5) * thT, with the 0.5*to coming from a
            0.5-scaled identity in the transpose. H2 (= 2h, for the output
            DMA) is computed off the recurrence chain.

            ptb: psum staging tile [128, 2, 4, BL] -- region [:,0] takes
            thT, region [:,1] takes 0.5*toT (reused from the attention
            staging, drained by then). hT_out(h) -> [128, 2, BL] dst slice.
            """
            tt = {0: t_i, 1: t_f, 2: t_g, 3: t_o}
            for ng in GORD:
                sc = 1.0 if ng == 2 else 0.5
                nc.scalar.activation(out=tt[ng], in_=zt[ng],
                                     func=AF.Tanh, bias=0.0, scale=sc)
            # fS emitted first: only depends on tanh_f (first tanh in GORD)
            nc.vector.scalar_tensor_tensor(out=fS, in0=t_f, scalar=1.0, in1=S,
                                           op0=OP.add, op1=OP.mult)
            if dummy_mm is not None:
                dummy_mm(t_f)
                dummy_mm(t_g)
            # b2 = (ti+1)*tg = 2ig
            nc.vector.scalar_tensor_tensor(out=b2, in0=t_i, scalar=1.0, in1=t_g,
                                           op0=OP.add, op1=OP.mult)
            if dummy_mm is not None:
                dummy_mm(fS)
            # 0.5*to transposed (off-chain: t_o ready right after its group);
            # staged to SBUF since the hT stt may read only one PSUM input.
            # The 0.5 folds into the staging copy (HW transpose mode does
            # not multiply by the identity's values).
            for ec in range(4):
                nc.tensor.transpose(ptb[:, 1, ec],
                                    t_o[:, ec * 128:(ec + 1) * 128], id4)
            nc.vector.tensor_scalar(out=toT, in0=ptb[:, 1], scalar1=0.5,
                                    scalar2=None, op0=OP.mult)
            for h in range(2):
                c = slice(h * 256, (h + 1) * 256)
                # S_new = 0.5*fS + b2 = 2c_new
                nc.vector.scalar_tensor_tensor(
                    out=S[:, c], in0=fS[:, c], scalar=0.5, in1=b2[:, c],
                    op0=OP.mult, op1=OP.add)
                nc.scalar.activation(out=th[:, c], in_=S[:, c], func=AF.Tanh,
                                     bias=0.0, scale=0.5)
                for ec in (2 * h, 2 * h + 1):
                    nc.tensor.transpose(ptb[:, 0, ec],
                                        th[:, ec * 128:(ec + 1) * 128], id4)
                # hT half = (0.5*to + 0.5) * th, transposed
                nc.vector.scalar_tensor_tensor(
                    out=hT_out(h), in0=toT[:, 2 * h:2 * h + 2], scalar=0.5,
                    in1=ptb[:, 0, 2 * h:2 * h + 2], op0=OP.add, op1=OP.mult)
            if dummy_mm is not None:
                dummy_mm(th)
            if emit_H2:
                for h in range(2):
                    c = slice(h * 256, (h + 1) * 256)
                    # H2 = (to+1)*th = 2h (output DMA only; off-chain)
                    nc.vector.scalar_tensor_tensor(
                        out=H2[:, c], in0=t_o[:, c], scalar=1.0, in1=th[:, c],
                        op0=OP.add, op1=OP.mult)

        # ===== PHASE 0: xwb = x @ Wih.T + enc_bias for all t (bf16) =====
        xwb = state.tile([128, 16 * G], BF16)
        with tc.tile_pool(name="p0", bufs=4, space="PSUM") as p0:
            for c in range(16):
                for ng in range(4):
                    sl = slice(ng * HE, (ng + 1) * HE)
                    pp0 = p0.tile([128, HE], F32, tag="pp0")
                    nc.tensor.matmul(pp0, onesr, encb[:, sl],
                                     start=True, stop=False)
                    nc.tensor.matmul(pp0, xT[:, 128 * c:128 * (c + 1)],
                                     wihT[:, sl], start=False, stop=True)
                    dst = xwb[:, c * G + ng * HE: c * G + (ng + 1) * HE]
                    if ng % 2 == 0:
                        nc.vector.tensor_copy(dst, pp0)
                    else:
                        nc.scalar.copy(dst, pp0)

        # ================= ENCODER =================
        with tc.tile_pool(name="eps", bufs=1, space="PSUM") as eps, \
             tc.tile_pool(name="ept", bufs=1, space="PSUM") as ept, \
             tc.tile_pool(name="edum", bufs=1, space="PSUM") as edum:
            dum_e = edum.tile([BL, 16], F32)
            # one PSUM bank staging the thT/toT transposes
            eptb = ept.tile([128, 2, 4, BL], F32)

            def enc_dummy(src):
                nc.tensor.matmul(dum_e, id4, src[:, 0:16], start=True, stop=True)

            def enc_inject(t):
                # xwb inject opens each z group; emitted at the END of step
                # t-1 so it schedules into the tail's PE idle (it has no
                # encT dependency, only the zt WAR on the gate tanh reads).
                zt = {ng: eps.tile([BL, HE], F32, tag=f"z{ng}", name=f"ez{ng}_{t}")
                      for ng in GORD}
                c0, r0 = t // 32, (t % 32) * BL
                for ng in GORD:
                    nc.tensor.matmul(zt[ng], id128[:, r0:r0 + BL],
                                     xwb[:, c0 * G + ng * HE: c0 * G + (ng + 1) * HE],
                                     start=True, stop=(t == 0))
                return zt

            zt = enc_inject(0)
            for t in range(nt):
                if t > 0:
                    for ng in GORD:
                        sl = slice(ng * HE, (ng + 1) * HE)
                        for kc in range(4):
                            nc.tensor.matmul(
                                zt[ng],
                                encT[:, kc, BL * (t - 1): BL * t],
                                whhT[:, kc * G + ng * HE: kc * G + (ng + 1) * HE],
                                start=False, stop=(kc == 3))
                # hT halves land straight into encT; the next step's h-pass
                # kc0/1 matmuls only RAW-wait on their own half
                lstm_tail(zt, S_enc, eptb,
                          lambda h, _t=t: encT[:, 2 * h:2 * h + 2,
                                               BL * _t: BL * (_t + 1)],
                          dummy_mm=enc_dummy)
                if t + 1 < nt:
                    zt = enc_inject(t + 1)

        # ============ PHASE 2: enc_projT and enc_tb ============
        with tc.tile_pool(name="p2a", bufs=4, space="PSUM") as p2a, \
             tc.tile_pool(name="p2b", bufs=4, space="PSUM") as p2b:
            for ac in range(2):
                for b in range(BL):
                    pp = p2a.tile([128, nt], F32, tag="pp")
                    for ec in range(4):
                        rhs = encT[:, ec, b: b + BL * (nt - 1) + 1: BL]
                        nc.tensor.matmul(
                            pp,
                            wencT[:, ec * A + ac * 128: ec * A + (ac + 1) * 128],
                            rhs, start=(ec == 0), stop=(ec == 3))
                    nc.scalar.activation(
                        out=enc_projT[:, ac * (nt * BL) + b * nt:
                                      ac * (nt * BL) + (b + 1) * nt],
                        in_=pp, func=AF.Identity, bias=epb[:, ac:ac + 1], scale=1.0)
            # enc_tb via PE transposes of encT (transpose keeps dtype)
            for b in range(BL):
                for tc_i in range(nt // 128):
                    for ec in range(4):
                        base = (128 * tc_i) * BL + b
                        src = encT[:, ec, base: base + BL * 127 + 1: BL]
                        pt2 = p2b.tile([128, 128], BF16, tag="pt2")
                        nc.tensor.transpose(pt2, src, id128)
                        nc.vector.tensor_copy(
                            enc_tb[:, (b * (nt // 128) + tc_i) * HE + ec * 128:
                                   (b * (nt // 128) + tc_i) * HE + (ec + 1) * 128],
                            pt2)

        # ================= DECODER =================
        with tc.tile_pool(name="dz", bufs=1, space="PSUM") as dz, \
             tc.tile_pool(name="dsp", bufs=1, space="PSUM") as dsp, \
             tc.tile_pool(name="dlc", bufs=1, space="PSUM") as dlc, \
             tc.tile_pool(name="dpt", bufs=1, space="PSUM") as dpt:
            # one PSUM bank holding 4 rotating [128, BL] transpose slots
            # one PSUM bank staging every transpose group; each group is
            # drained by one (or two) strided DVE copies, so the PE never
            # ping-pongs with the DVE on single-tile WAR chains.
            # region [:, 0]    : hdT staging  (ec, b)
            # region [:, 1, :, 0:2]: w-pair staging (tc, b%2)
            # region [:, 1, :, 2:4]: ctxT-pair staging (ec, b%2)
            dptb = dpt.tile([128, 2, 4, BL], F32)

            def align_op(b, ac):
                s = (b % 2) * 2 + ac
                nc.scalar.activation(
                    out=align[:, s * T: s * T + nt],
                    in_=enc_projT[:, ac * (nt * BL) + b * nt:
                                  ac * (nt * BL) + (b + 1) * nt],
                    func=AF.Tanh, bias=dsT[:, ac * BL + b: ac * BL + b + 1],
                    scale=1.0)

            def vred_op(lg_ps, b, ac):
                # lhsT [128,2] has V(ac) at col b%2, zeros at the other, so
                # logits(b) land in pair row b%2; the pair's 4 mms form one
                # accumulation group (rows take turns getting +0).
                col = (b * 2 + ac) * BL + (b & ~1)
                s = (b % 2) * 2 + ac
                nc.tensor.matmul(
                    lg_ps[:, :nt],
                    vblk[:, col:col + 2],
                    align[:, s * T: s * T + nt],
                    start=(b % 2 == 0 and ac == 0), stop=(b % 2 == 1 and ac == 1))

            def hp_op(zt, ng):
                for kc in range(4):
                    nc.tensor.matmul(
                        zt[ng],
                        hdT[:, kc],
                        dhr[:, kc * G + ng * HD: kc * G + (ng + 1) * HD],
                        start=False, stop=False)

            def exp_op(lg_ps, p):
                nc.scalar.activation(
                    out=w_u[p][:, :nt],
                    in_=lg_ps[:, :nt], func=AF.Exp,
                    bias=0.0, scale=1.0, accum_out=denom[p])

            def wtr_op(p):
                # transpose the pair's (unnormalized) softmax rows into the
                # staging bank, then two strided copies into wTb slots 0/3
                for tc_i in range(nt // 128):
                    nc.tensor.transpose(
                        dptb[:, 1, tc_i, 0:2],
                        w_u[p][:, tc_i * 128:(tc_i + 1) * 128],
                        id4[0:2, 0:2])
                nc.vector.tensor_copy(wTb[:, p, :, 0:4:3], dptb[:, 1, :, 0:2])

            def ctx_op(ctx_ps, p, rows=(0, 1)):
                # one accumulation group per pair; rows may be emitted in
                # two chunks with other matmuls interleaved between
                for s in rows:
                    b = 2 * p + s
                    for tc_i in range(nt // 128):
                        base = (b * (nt // 128) + tc_i) * HE
                        nc.tensor.matmul(
                            ctx_ps[:, :HE],
                            wTb[:, p, tc_i, s: s + 3: 2],
                            enc_tb[:, base: base + HE],
                            start=(s == 0 and tc_i == 0),
                            stop=(s == 1 and tc_i == 3))

            def norm_op(ctx_ps, p, split=False):
                if split:
                    # halves in parallel on DVE + ACT (~400ns vs 658)
                    nc.vector.tensor_scalar(out=cs[p][:, 0:HE // 2],
                                            in0=ctx_ps[:, 0:HE // 2],
                                            scalar1=rcp[p],
                                            scalar2=None, op0=OP.mult)
                    nc.scalar.mul(cs[p][:, HE // 2:HE], ctx_ps[:, HE // 2:HE],
                                  rcp[p])
                    return
                nc.vector.tensor_scalar(out=cs[p], in0=ctx_ps[:, :HE],
                                        scalar1=rcp[p],
                                        scalar2=None, op0=OP.mult)

            def ctxT_op(p, h=None):
                ecs = range(4) if h is None else (2 * h, 2 * h + 1)
                for ec in ecs:
                    nc.tensor.transpose(
                        dptb[:, 1, ec, 2:4], cs[p][:, ec * 128:(ec + 1) * 128],
                        id4[0:2, 0:2])
                e0, e1 = (0, 4) if h is None else (2 * h, 2 * h + 2)
                nc.vector.tensor_copy(ctxT[:, e0:e1, 2 * p:2 * p + 2],
                                      dptb[:, 1, e0:e1, 2:4])

            def dec_bias(t):
                # bias matmuls open each z group; emitted at the END of step
                # t-1 so they schedule into the tail's PE idle.
                zt = {ng: dz.tile([BL, HD], F32, tag=f"zd{ng}", name=f"dz{ng}_{t}")
                      for ng in GORD}
                for ng in GORD:
                    nc.tensor.matmul(zt[ng], ones, decb[:, ng * HD:(ng + 1) * HD],
                                     start=True, stop=False)
                return zt

            zt = dec_bias(0)
            for t in range(nt):
                dsT_ps = dsp.tile([128, 2 * BL], F32, tag="dsT")
                lg_ps = dlc.tile([2, T], F32, tag="lg")
                ctx_ps = dlc.tile([2, T], F32, tag="cx")

                def dec_dummy(src, _p=dsT_ps):
                    nc.tensor.matmul(_p[0:BL, 0:8], id4, src[:, 0:8],
                                     start=True, stop=True)

                # --- dscore, directly transposed: dsT = Wdec @ h ---
                for ac in range(2):
                    for kc in range(4):
                        nc.tensor.matmul(
                            dsT_ps[:, ac * BL:(ac + 1) * BL],
                            wdT[:, kc * A + ac * 128: kc * A + (ac + 1) * 128],
                            hdT[:, kc],
                            start=(kc == 0), stop=(kc == 3))
                nc.vector.tensor_copy(dsT, dsT_ps)
                # --- attention pipelined per pair of batch rows; the h-pass
                # matmuls stream inside the align-tanh window. Pair 0's
                # softmax/ctx/normalize/ctxT all run inside pair 1's align
                # window; only pair 1's trail is on the serial chain. ---
                for b in (0, 1):
                    align_op(b, 0)
                    align_op(b, 1)
                hp_op(zt, GORD[0])
                vred_op(lg_ps, 0, 0)
                vred_op(lg_ps, 0, 1)
                hp_op(zt, GORD[1])
                vred_op(lg_ps, 1, 0)
                vred_op(lg_ps, 1, 1)
                exp_op(lg_ps, 0)
                for b in (2, 3):
                    align_op(b, 0)
                    align_op(b, 1)
                vred_op(lg_ps, 2, 0)
                vred_op(lg_ps, 2, 1)
                wtr_op(0)
                nc.vector.reciprocal(rcp[0], denom[0])
                ctx_op(ctx_ps, 0, rows=(0,))
                vred_op(lg_ps, 3, 0)
                vred_op(lg_ps, 3, 1)
                ctx_op(ctx_ps, 0, rows=(1,))
                exp_op(lg_ps, 1)
                wtr_op(1)
                nc.vector.reciprocal(rcp[1], denom[1])
                norm_op(ctx_ps, 0)
                ctx_op(ctx_ps, 1)
                ctxT_op(0)
                hp_op(zt, GORD[2])
                hp_op(zt, GORD[3])
                norm_op(ctx_ps, 1)
                ctxT_op(1)
                # --- ctx-pass, grouped per gate; tail follows early ---
                for ng in GORD:
                    sl = slice(ng * HD, (ng + 1) * HD)
                    for kc in range(4):
                        nc.tensor.matmul(
                            zt[ng],
                            ctxT[:, kc],
                            dcr[:, kc * G + ng * HD: kc * G + (ng + 1) * HD],
                            start=False, stop=(kc == 3))
                # --- gates + state; transposed h lands straight in hdT ---
                lstm_tail(zt, S_dec, dptb, lambda h: hdT[:, 2 * h:2 * h + 2],
                          dummy_mm=dec_dummy, emit_H2=True)
                # --- output h (as 2h; host rescales) ---
                nc.sync.dma_start(out=d_out.ap()[:, t, :], in_=H2)
                if t + 1 < nt:
                    zt = dec_bias(t + 1)

    split_multi_waits(nc)
    return nc


# ----------------------------------------------------------------------
def _sig(x):
    return 1.0 / (1.0 + np.exp(-x))


def prepare_inputs(inputs, nt=T):
    """Host-side weight/layout prep. Returns per_core_fn."""
    f32 = np.float32
    enc_Wih = np.asarray(inputs["enc_Wih"], f32)
    enc_Whh = np.asarray(inputs["enc_Whh"], f32)
    enc_bias = np.asarray(inputs["enc_bih"], f32) + np.asarray(inputs["enc_bhh"], f32)
    Wenc_w = np.asarray(inputs["Wenc_w"], f32)
    Wenc_b = np.asarray(inputs["Wenc_b"], f32)
    Wdec_w = np.asarray(inputs["Wdec_w"], f32)
    Wdec_b = np.asarray(inputs["Wdec_b"], f32)
    V_w = np.asarray(inputs["V_w"], f32)
    attn_bias = np.asarray(inputs["attn_bias"], f32)
    dec_Wih = np.asarray(inputs["dec_Wih"], f32)
    dec_Whh = np.asarray(inputs["dec_Whh"], f32)
    dec_bias = np.asarray(inputs["dec_bih"], f32) + np.asarray(inputs["dec_bhh"], f32)

    sh = {}
    sh["whhT"] = np.ascontiguousarray(
        enc_Whh.T.reshape(4, 128, G)).astype(bf16)
    sh["wihT"] = np.ascontiguousarray(enc_Wih.T).astype(bf16)
    sh["encb"] = enc_bias.reshape(1, -1).astype(bf16)
    dec_h_w = (dec_Wih[:, HD:] + dec_Whh)           # [2048, 512]
    sh["dhr"] = np.ascontiguousarray(
        dec_h_w.T.reshape(4, 128, G)).astype(bf16)
    sh["dcr"] = np.ascontiguousarray(
        dec_Wih[:, :HD].T.reshape(4, 128, G)).astype(bf16)
    # wdT[kc][k, ac*128+m] = Wdec_w[ac*128+m, kc*128+k]
    sh["wdT"] = np.ascontiguousarray(
        Wdec_w.T.reshape(4, 128, A)).astype(bf16)
    sh["decb"] = dec_bias.reshape(1, -1).astype(bf16)
    sh["wencT"] = np.ascontiguousarray(
        Wenc_w.T.reshape(4, 128, A)).astype(bf16)
    sh["epb"] = np.ascontiguousarray(
        (Wenc_b + attn_bias + Wdec_b).reshape(2, 128, 1)).astype(f32)
    vb = np.zeros((128, 8 * BL), f32)
    for b in range(BL):
        for ac in range(2):
            vb[:, (b * 2 + ac) * BL + b] = V_w[0, ac * 128:(ac + 1) * 128]
    sh["vblk"] = vb.astype(bf16)
    sh["ones"] = np.ones((1, BL), f32).astype(bf16)
    sh["onesr"] = np.ones((1, 128), f32).astype(bf16)
    sh["id4"] = np.eye(BL, dtype=f32)
    sh["id4h"] = 0.5 * np.eye(BL, dtype=f32)
    sh["id128"] = np.eye(128, dtype=f32).astype(bf16)
    # decoder init state (z0 from biases only)
    i0, f0, g0, o0 = np.split(dec_bias, 4)
    cd0 = _sig(i0) * np.tanh(g0)
    hd0 = _sig(o0) * np.tanh(cd0)
    hd0T = np.zeros((128, 4, BL), f32)
    for ec in range(4):
        for b in range(BL):
            hd0T[:, ec, b] = hd0[ec * 128:(ec + 1) * 128]
    sh["hd0T"] = hd0T.astype(bf16)
    sh["S0"] = np.broadcast_to(2.0 * cd0, (BL, HD)).astype(f32).copy()

    x = np.asarray(inputs["x"], f32)

    def core_inputs(core):
        xc = x[core * BL:(core + 1) * BL, :nt, :]      # [BL, nt, F]
        xT = np.ascontiguousarray(xc.transpose(2, 1, 0).reshape(128, nt * BL))
        m = dict(sh)
        m["xT"] = xT.astype(bf16)
        return m

    return core_inputs


_cache = {}


def kernel(**inputs):
    nt = np.asarray(inputs["x"]).shape[1]
    if nt not in _cache:
        _cache[nt] = build_nc(nt)
    nc = _cache[nt]
    core_inputs = prepare_inputs(inputs, nt)
    in_maps = [core_inputs(c) for c in range(NCORES)]
    res = run_bass_kernel_spmd(nc, in_maps, core_ids=list(range(NCORES)))
    outs = [res.results[c]["out"] for c in range(NCORES)]
    full = np.concatenate(outs, axis=0) * 0.5
    return full.astype(np.float32)

